# revision 6
# baseline (speedup 1.0000x reference)
"""LSTM layer kernel for Trainium2, 8 NeuronCores, batch-data-parallel.

Per core (batch shard of 8):
  xg = input @ W_ih.T + bias_ih + bias_hh            (fp32, per S-quarter)
  per step: gates^T = W_hh @ h  (+ xg[t]); i,f,g,o; c = f*c+i*g; h = o*tanh(c)

Layout: "gates transposed" — hidden/gate units on partitions, (strip, batch)
on the free dim.  W_hh^T tiles are the stationary operand (bf16) with h^T
chunks moving; the epilogue is partition-dense [128, 32]/[128, 128] ops, and
h is produced directly in the layout the next step's matmul consumes.
All DMAs are contiguous-last-dim; layout changes happen via PE transposes.
The recurrence runs in a hardware For_i loop to keep the NEFF small.
"""

import sys

sys.path.insert(0, "/opt/trn_rl_repo")

import numpy as np

S_FULL = 512
BL = 8          # batch per core
H = 512
I_DIM = 512
G = 2048        # 4*H
KC = 4          # contraction chunks of 128
MS = 16         # gate-dim strips of 128

# strip order i(0-3), f(4-7), o(8-11), g(12-15); dram row order is i,f,g,o
PR = ([0 + 128 * j for j in range(4)]
      + [512 + 128 * j for j in range(4)]
      + [1536 + 128 * j for j in range(4)]
      + [1024 + 128 * j for j in range(4)])


def build(S=S_FULL, SQ=128):
    from concourse import bass, bacc, mybir
    from concourse.tile import TileContext

    NQ = S // SQ
    assert NQ * SQ == S and SQ % 4 == 0
    CS = min(512, SQ * BL)          # phase-1 moving chunk
    NSBC = (SQ * BL) // CS
    f32 = mybir.dt.float32
    bf16 = mybir.dt.bfloat16
    i32 = mybir.dt.int32
    AF = mybir.ActivationFunctionType

    nc = bacc.Bacc("TRN2", target_bir_lowering=False, debug=False, num_devices=8)

    inp = nc.dram_tensor("input", [S, BL, I_DIM], f32, kind="ExternalInput")
    w_ih = nc.dram_tensor("weight_ih", [G, I_DIM], f32, kind="ExternalInput")
    w_hh = nc.dram_tensor("weight_hh", [G, H], f32, kind="ExternalInput")
    b_ih = nc.dram_tensor("bias_ih", [G], f32, kind="ExternalInput")
    b_hh = nc.dram_tensor("bias_hh", [G], f32, kind="ExternalInput")
    outp = nc.dram_tensor("out", [S, BL, H], f32, kind="ExternalOutput")
    c_out = nc.dram_tensor("c_out", [BL, H], f32, kind="ExternalOutput")

    inp_flat = inp.rearrange("s b i -> (s b) i")

    wihT = nc.alloc_sbuf_tensor("wihT", [128, KC * G], f32)
    whhT = nc.alloc_sbuf_tensor("whhT", [128, KC * G], bf16)
    xg_q = nc.alloc_sbuf_tensor("xg_q", [128, SQ * 128], f32)
    in_t = nc.alloc_sbuf_tensor("in_t", [128, KC, SQ * BL], f32)
    h_hist = nc.alloc_sbuf_tensor("h_hist", [128, SQ * 32], f32)
    h_bf = nc.alloc_sbuf_tensor("h_bf", [128, 32], bf16)
    c_sb = nc.alloc_sbuf_tensor("c_sb", [128, 32], f32)
    gs = nc.alloc_sbuf_tensor("gs", [128, 128], f32)
    ig = nc.alloc_sbuf_tensor("ig", [128, 32], f32)
    tc_t = nc.alloc_sbuf_tensor("tc_t", [128, 32], f32)
    bias_s = nc.alloc_sbuf_tensor("bias_s", [1, G], f32)
    bias_t = nc.alloc_sbuf_tensor("bias_t", [1, G], f32)
    ones = nc.alloc_sbuf_tensor("ones", [1, CS], f32)
    id_i = nc.alloc_sbuf_tensor("id_i", [128, 128], i32)
    id_f = nc.alloc_sbuf_tensor("id_f", [128, 128], f32)

    psum_g = nc.alloc_psum_tensor("psum_g", [128, 128], f32)
    psum_g2 = nc.alloc_psum_tensor("psum_g2", [128, 128], f32)
    psum_x = nc.alloc_psum_tensor("psum_x", [128, CS], f32)
    psum_t = nc.alloc_psum_tensor("psum_t", [128, 128], f32)

    xgv = xg_q.ap().rearrange("p (t m b) -> p t m b", m=MS, b=BL)
    hhv = h_hist.ap().rearrange("p (t k b) -> p t k b", k=KC, b=BL)

    with TileContext(nc) as tc:
        with tc.tile_pool(name="wst", bufs=3) as wpool:
            # identity for PE transposes: id[p, j] = (j - p == 0)
            nc.gpsimd.iota(id_i[:, :], [[1, 128]], channel_multiplier=-1)
            nc.vector.tensor_scalar(id_f[:, :], id_i[:, :], 0, None,
                                    mybir.AluOpType.is_equal)

            # weight preload: contiguous row-loads, PE-transpose to lhsT layout
            for m in range(MS):
                wst = wpool.tile([128, 512], f32, tag="wst")
                nc.sync.dma_start(out=wst[:, :], in_=w_ih[PR[m]:PR[m] + 128, :])
                for k in range(KC):
                    nc.tensor.transpose(psum_t[:, :], wst[:, 128 * k:128 * (k + 1)],
                                        id_f[:, :])
                    nc.vector.tensor_copy(
                        wihT[:, k * G + 128 * m: k * G + 128 * (m + 1)], psum_t[:, :])
            for m in range(MS):
                wst = wpool.tile([128, 512], f32, tag="wst")
                nc.sync.dma_start(out=wst[:, :], in_=w_hh[PR[m]:PR[m] + 128, :])
                for k in range(KC):
                    nc.tensor.transpose(psum_t[:, :], wst[:, 128 * k:128 * (k + 1)],
                                        id_f[:, :])
                    nc.vector.tensor_copy(
                        whhT[:, k * G + 128 * m: k * G + 128 * (m + 1)], psum_t[:, :])
            for m in range(MS):
                nc.sync.dma_start(out=bias_s[0:1, 128 * m:128 * (m + 1)],
                                  in_=b_ih[PR[m]:PR[m] + 128])
                nc.sync.dma_start(out=bias_t[0:1, 128 * m:128 * (m + 1)],
                                  in_=b_hh[PR[m]:PR[m] + 128])
            nc.vector.tensor_add(bias_s[0:1, :], bias_s[0:1, :], bias_t[0:1, :])
            nc.vector.memset(ones[0:1, :], 1.0)
            nc.vector.memset(h_bf[:, :], 0.0)
            nc.vector.memset(c_sb[:, :], 0.0)

            rows_q = SQ * BL
            for q in range(NQ):
                s0 = q * SQ
                # ---- phase 1: x-gates for this quarter (fp32) ----
                for r in range((rows_q + 127) // 128):
                    rr = min(128, rows_q - 128 * r)
                    ist = wpool.tile([128, 512], f32, tag="ist")
                    nc.sync.dma_start(
                        out=ist[:rr, :],
                        in_=inp_flat[s0 * BL + 128 * r: s0 * BL + 128 * r + rr, :])
                    for k in range(KC):
                        nc.tensor.transpose(psum_t[:, :rr],
                                            ist[:rr, 128 * k:128 * (k + 1)],
                                            id_f[:rr, :rr])
                        nc.vector.tensor_copy(
                            in_t[:, k, 128 * r:128 * r + rr], psum_t[:, :rr])
                for m in range(MS):
                    for sc in range(NSBC):
                        for k in range(KC):
                            nc.tensor.matmul(
                                psum_x[:, :],
                                wihT[:, k * G + 128 * m: k * G + 128 * (m + 1)],
                                in_t[:, k, sc * CS:(sc + 1) * CS],
                                start=(k == 0), stop=False)
                        nc.tensor.matmul(
                            psum_x[:, :],
                            bias_s[0:1, 128 * m:128 * (m + 1)],
                            ones[0:1, :],
                            start=False, stop=True)
                        tq = sc * (CS // BL)
                        nc.vector.tensor_copy(
                            xgv[:, tq:tq + CS // BL, m, :],
                            psum_x[:, :].rearrange("p (t b) -> p t b", b=BL))

                # ---- phase 2: recurrence over SQ steps (2x unrolled) ----
                with tc.For_i(0, SQ, 2) as t:
                    for u, pg in ((0, psum_g), (1, psum_g2)):
                        for m in range(MS):
                            for k in range(KC):
                                nc.tensor.matmul(
                                    pg[:, 8 * m:8 * m + 8],
                                    whhT[:, k * G + 128 * m: k * G + 128 * (m + 1)],
                                    h_bf[:, 8 * k:8 * k + 8],
                                    start=(k == 0), stop=(k == 3))
                        nc.vector.tensor_add(gs[:, :], pg[:, :],
                                             xg_q[:, bass.ds(t * 128 + u * 128, 128)])
                        nc.scalar.activation(gs[:, 0:64], gs[:, 0:64], AF.Sigmoid)
                        nc.scalar.activation(gs[:, 96:128], gs[:, 96:128], AF.Tanh)
                        nc.vector.tensor_mul(ig[:, :], gs[:, 0:32], gs[:, 96:128])
                        nc.vector.tensor_mul(c_sb[:, :], gs[:, 32:64], c_sb[:, :])
                        nc.vector.tensor_add(c_sb[:, :], c_sb[:, :], ig[:, :])
                        nc.scalar.activation(gs[:, 64:96], gs[:, 64:96], AF.Sigmoid)
                        nc.scalar.activation(tc_t[:, :], c_sb[:, :], AF.Tanh)
                        nc.vector.tensor_mul(h_bf[:, :], gs[:, 64:96], tc_t[:, :])
                        nc.gpsimd.tensor_mul(h_hist[:, bass.ds(t * 32 + u * 32, 32)],
                                             gs[:, 64:96], tc_t[:, :])

                # ---- transpose h history to [batch-major] and flush ----
                for g4 in range(SQ // 4):
                    ost = wpool.tile([128, 128], f32, tag="ost")
                    nc.tensor.transpose(psum_t[:, :],
                                        h_hist[:, 128 * g4:128 * (g4 + 1)],
                                        id_f[:, :])
                    nc.vector.tensor_copy(ost[:, :], psum_t[:, :])
                    for t4 in range(4):
                        dst = outp[s0 + 4 * g4 + t4, :, :].rearrange(
                            "b (k p) -> k b p", k=KC, p=128)
                        nc.sync.dma_start(out=dst, in_=ost[32 * t4:32 * t4 + 32, :])

            # ---- final c ----
            nc.tensor.transpose(psum_t[0:32, :], c_sb[:, :], id_f[:, :])
            cst = wpool.tile([32, 128], f32, tag="cst")
            nc.vector.tensor_copy(cst[:, :], psum_t[0:32, :])
            nc.sync.dma_start(
                out=c_out.rearrange("b (k p) -> k b p", k=KC, p=128),
                in_=cst[:, :])

    nc.compile()
    return nc


_NC_CACHE = {}


def kernel(input, weight_ih, weight_hh, bias_ih, bias_hh):
    from concourse.bass_utils import run_bass_kernel_spmd

    if "full" not in _NC_CACHE:
        _NC_CACHE["full"] = build()
    nc = _NC_CACHE["full"]

    input = np.ascontiguousarray(np.asarray(input, dtype=np.float32))
    in_maps = []
    for c in range(8):
        in_maps.append({
            "input": np.ascontiguousarray(input[:, 8 * c:8 * c + 8, :]),
            "weight_ih": np.asarray(weight_ih, np.float32),
            "weight_hh": np.asarray(weight_hh, np.float32),
            "bias_ih": np.asarray(bias_ih, np.float32),
            "bias_hh": np.asarray(bias_hh, np.float32),
        })
    res = run_bass_kernel_spmd(nc, in_maps, core_ids=list(range(8)))
    outputs = np.concatenate([res.results[c]["out"] for c in range(8)], axis=1)
    c_fin = np.concatenate([res.results[c]["c_out"] for c in range(8)], axis=0)
    h_fin = outputs[-1].copy()
    return outputs, h_fin, c_fin
